# revision 1
# baseline (speedup 1.0000x reference)
"""GAT (2-layer graph attention) Trainium2 kernel.

Sharding (SPMD, 8 cores): batch b = core//2; within a core pair, the 4
attention heads are split 2+2. Per-core differences are pure data (packed
per-core weights); the program is identical on all cores. Between layers,
pair-local collectives stitch the halves: AllGather for the layer-0
head-concat output, AllReduce(add) for the layer-1 head-mean.

Score tiles are computed in transposed layout [j_part, i_free] so the
aggregation matmul contracts j on partitions with the probability tile as
the PE moving operand (no P transposes). Softmax skips max-subtraction
(scores verified |s| <= ~6.1 on the real data), masks multiplicatively
with a host-prepped transposed {0,1} bf16 mask that already includes
self-loops, and folds the softmax normalizer Z into the aggregation via a
ones column appended to the source features.

Per layer, the two local heads use different engine paths so ACT / DVE /
PE all stay busy:
  head A (ACT-heavy):  s = Prelu(e_dst_bcast + e_src) on ACT, t = Exp(s)
      on ACT, P = t * mask on DVE, one PE stream P^T @ [x0|1].
  head C (no ACT):  exp(leaky(u)) factorizes per branch of the leaky:
      P = g*H*e^{ed_i}e^{es_j} + g*(1-H)*e^{.2ed_i}e^{.2es_j},
      H = 1{u>=0} via one DVE compare (vs -e_src), G1 = g*H via one DVE
      mult. PE streams: G1 against stacked weights [x0*e^{es}|x0*e^{.2es}]
      (M1 rows 0:33, M2 rows 33:66) and g against x0*e^{.2es} (Mg). Then
      out = e^{ed}.M1 + e^{.2ed}.(Mg - M2), applied per-partition after
      the transpose back to node-major layout.

Host-side prep (not in HW time): slice x_alpha[:,-1], transpose+augment x,
build per-batch transposed masks, pack/fold weights per core.
"""
import numpy as np
import ml_dtypes
from contextlib import ExitStack

import concourse.bass as bass
import concourse.mybir as mybir
import concourse.tile as tile
from concourse import bacc
from concourse.bass_utils import run_bass_kernel_spmd
from concourse.masks import make_identity

F32 = mybir.dt.float32
BF16 = mybir.dt.bfloat16
AF = mybir.ActivationFunctionType
ALU = mybir.AluOpType

B, T, N, F_IN = 4, 8, 2000, 158
D, H, C = 128, 4, 32
HL = 2           # heads per core
NB = 16          # node blocks
TB = 125         # nodes per block
FA = F_IN + 1    # augmented features (ones col carries b_in)
KA = 128
KB = FA - KA     # 31
PW = HL * C + 2 * HL  # packed stage-1 cols: x0 (2 heads), e_src, e_dst
NEG_SLOPE = 0.2
AGG_DT = BF16
# aggregation i-chunks: PSUM zero regions are 2KB banks, so accumulation
# chunks must start at 512-f32 boundaries
QF = [(0, 512), (512, 1024), (1024, 1536), (1536, 2000)]        # full-i
QH = [[(0, 512), (512, 1024)], [(1024, 1536), (1536, 2000)]]    # i-halves

_CACHE = {}


def ts(i, n):
    return slice(i * n, (i + 1) * n)


def _build_program(dumps=False, no_cc=False, c_heads=(1,)):
    nc = bacc.Bacc("TRN2", target_bir_lowering=False, debug=False, num_devices=8)

    xTa_d = nc.dram_tensor("xTa", [KA, N], F32, kind="ExternalInput")
    xTb_d = nc.dram_tensor("xTb", [KB, N], F32, kind="ExternalInput")
    gT_d = nc.dram_tensor("gT", [N, N], BF16, kind="ExternalInput")
    WiaA_d = nc.dram_tensor("WiaA", [KA, D], F32, kind="ExternalInput")
    WiaB_d = nc.dram_tensor("WiaB", [KB, D], F32, kind="ExternalInput")
    Wp0_d = nc.dram_tensor("Wp0", [D, PW], F32, kind="ExternalInput")
    Wp1_d = nc.dram_tensor("Wp1", [D, PW], F32, kind="ExternalInput")
    Wad0_d = nc.dram_tensor("Wad0", [D, HL], F32, kind="ExternalInput")
    Wad1_d = nc.dram_tensor("Wad1", [D, HL], F32, kind="ExternalInput")
    lngb_d = nc.dram_tensor("lngb", [128, D], F32, kind="ExternalInput")
    lnbb_d = nc.dram_tensor("lnbb", [128, D], F32, kind="ExternalInput")
    b0b_d = nc.dram_tensor("b0b", [128, D], F32, kind="ExternalInput")
    Woa_d = nc.dram_tensor("Woa", [C + 1, D], F32, kind="ExternalInput")
    out_d = nc.dram_tensor("out", [N, D], F32, kind="ExternalOutput")
    dbg = {}
    if dumps:
        for nm, shp in (("hN", [TB, NB, D]), ("agin", [N, HL * C]),
                        ("hN2", [TB, NB, D]), ("arin", [N, C]),
                        ("arout", [N, C])):
            dbg[nm] = nc.dram_tensor(f"dbg_{nm}", shp, F32, kind="ExternalOutput")

    PAIRS = [[0, 1], [2, 3], [4, 5], [6, 7]]

    with tile.TileContext(nc) as tc, ExitStack() as ctx:
        persist = ctx.enter_context(tc.tile_pool(name="persist", bufs=1))
        work = ctx.enter_context(tc.tile_pool(name="work", bufs=3))
        sc_s = ctx.enter_context(tc.tile_pool(name="sc_s", bufs=1))
        sc_t = ctx.enter_context(tc.tile_pool(name="sc_t", bufs=2))
        sc_p = ctx.enter_context(tc.tile_pool(name="sc_p", bufs=2))
        ot_p = ctx.enter_context(tc.tile_pool(name="ot", bufs=1))
        psum = ctx.enter_context(tc.tile_pool(name="ps", bufs=2, space="PSUM"))
        psum_ag = ctx.enter_context(tc.tile_pool(name="psag", bufs=1, space="PSUM"))
        dram = ctx.enter_context(tc.tile_pool(name="dram", bufs=1, space="DRAM"))

        # ---- constants ----
        ident = persist.tile([128, 128], F32)
        make_identity(nc, ident)
        xTa = persist.tile([KA, N], F32)
        xTb = persist.tile([KB, N], F32)
        WiaA = persist.tile([KA, D], F32)
        WiaB = persist.tile([KB, D], F32)
        Wp = [persist.tile([D, PW], F32, name=f"Wp{l}", tag=f"Wp{l}")
              for l in range(2)]
        Wad = [persist.tile([D, HL], F32, name=f"Wad{l}", tag=f"Wad{l}")
               for l in range(2)]
        lngb = persist.tile([128, D], F32)
        lnbb = persist.tile([128, D], F32)
        b0b = persist.tile([128, D], F32)
        Woa = persist.tile([C + 1, D], F32)
        for sb, dr in ((xTa, xTa_d), (xTb, xTb_d), (WiaA, WiaA_d), (WiaB, WiaB_d),
                       (Wp[0], Wp0_d), (Wp[1], Wp1_d), (Wad[0], Wad0_d),
                       (Wad[1], Wad1_d), (lngb, lngb_d), (lnbb, lnbb_d),
                       (b0b, b0b_d), (Woa, Woa_d)):
            nc.sync.dma_start(out=sb[:], in_=dr[:])

        gT = persist.tile([TB, NB, N], BF16)
        for jb in range(NB):
            nc.sync.dma_start(out=gT[:, jb, :], in_=gT_d[ts(jb, TB), :])

        # ---- persistent activations ----
        eps_t = persist.tile([TB, 1], F32)
        nc.vector.memset(eps_t[:], 1e-5)
        hN = persist.tile([TB, NB, D], F32)
        hT = persist.tile([D, N], F32)
        x0ext = persist.tile([TB, NB, HL, C + 1], AGG_DT)
        es_N = persist.tile([TB, NB, HL], F32)
        esn = persist.tile([TB, NB, HL], F32)     # -e_src (compare scalar)
        edN = persist.tile([TB, NB, HL], F32)     # e_dst, node-major
        a1 = persist.tile([TB, NB, HL], F32)      # exp(e_dst)
        a2 = persist.tile([TB, NB, HL], F32)      # exp(0.2 e_dst)
        a2n = persist.tile([TB, NB, HL], F32)     # -exp(0.2 e_dst)
        esx1 = persist.tile([TB, NB, HL], BF16)   # exp(e_src)
        esx2 = persist.tile([TB, NB, HL], BF16)   # exp(0.2 e_src)
        xB12 = persist.tile([TB, NB, 2 * (C + 1)], AGG_DT)
        edTh = [persist.tile([1, N], F32, name=f"edTh{h}") for h in range(HL)]
        edb = persist.tile([128, HL, N], F32)
        h1aug = persist.tile([TB, NB, C + 1], F32)
        hpacc = persist.tile([TB, NB, C], F32)

        ag_in_d = dram.tile([N, HL * C], F32, tag="ag_in")
        ag_out_d = dram.tile([2, N, HL * C], F32, tag="ag_out")
        ar_in_d = dram.tile([N, C], F32, tag="ar_in")
        ar_out_d = dram.tile([N, C], F32, tag="ar_out")

        for jb in range(NB):
            nc.vector.memset(x0ext[:, jb, :, C:C + 1], 1.0)
            nc.vector.memset(h1aug[:, jb, C:C + 1], 1.0)

        # ---- stage 0: input projection + LN + ReLU, transpose to hT ----
        for nb in range(NB):
            ph = psum.tile([TB, D], F32, tag="ps")
            nc.tensor.matmul(ph[:], xTa[:, ts(nb, TB)], WiaA[:], start=True,
                             stop=False)
            nc.tensor.matmul(ph[:], xTb[:, ts(nb, TB)], WiaB[:], start=False,
                             stop=True)
            stats = work.tile([TB, 6], F32, tag="stats")
            nc.vector.bn_stats(out=stats[:], in_=ph[:])
            mv = work.tile([TB, 2], F32, tag="mv")
            nc.vector.bn_aggr(out=mv[:], in_=stats[:])
            sd = work.tile([TB, 1], F32, tag="sd")
            nc.scalar.activation(sd[:], mv[:, 1:2], AF.Sqrt, bias=eps_t[:, 0:1])
            rstd = work.tile([TB, 1], F32, tag="rstd")
            nc.vector.reciprocal(rstd[:], sd[:])
            hn = work.tile([TB, D], F32, tag="hn")
            nc.vector.tensor_scalar(out=hn[:], in0=ph[:], scalar1=mv[:, 0:1],
                                    scalar2=rstd[:, 0:1], op0=ALU.subtract,
                                    op1=ALU.mult)
            hg = work.tile([TB, D], F32, tag="hg")
            nc.vector.tensor_tensor(out=hg[:], in0=hn[:], in1=lngb[0:TB, :],
                                    op=ALU.mult)
            hb = work.tile([TB, D], F32, tag="hb")
            nc.vector.tensor_tensor(out=hb[:], in0=hg[:], in1=lnbb[0:TB, :],
                                    op=ALU.add)
            nc.vector.tensor_scalar(out=hN[:, nb, :], in0=hb[:], scalar1=0.0,
                                    scalar2=None, op0=ALU.max)
            pt = psum.tile([D, TB], F32, tag="ps")
            nc.tensor.transpose(pt[:], hN[:, nb, :], ident[0:TB, 0:TB])
            nc.scalar.copy(hT[:, ts(nb, TB)], pt[:])
        if dumps:
            nc.sync.dma_start(out=dbg["hN"][:], in_=hN[:])

        def stage1_block(l, hT_in, nb):
            px = psum.tile([TB, PW], F32, tag="ps")
            nc.tensor.matmul(px[:], hT_in[:, ts(nb, TB)], Wp[l][:],
                             start=True, stop=True)
            nc.vector.tensor_copy(
                x0ext[:, nb, :, 0:C],
                px[:, 0:HL * C].rearrange("p (h c) -> p h c", h=HL))
            nc.scalar.copy(es_N[:, nb, :], px[:, HL * C:HL * C + HL])
            nc.scalar.copy(edN[:, nb, :], px[:, HL * C + HL:PW])

        def stage1(l, hT_in, skip_px=False):
            """x0ext / e_src / e_dst and all per-layer C-path prep."""
            if not skip_px:
                for nb in range(NB):
                    stage1_block(l, hT_in, nb)
            # e_dst in T-layout: edT_h = (W Ad_h).T @ hT
            for q in range(4):
                for h in range(HL):
                    pe = psum.tile([1, 500], F32, tag="ps")
                    nc.tensor.matmul(pe[:], Wad[l][:, h:h + 1],
                                     hT_in[:, ts(q, 500)], start=True, stop=True)
                    nc.vector.tensor_copy(edTh[h][0:1, ts(q, 500)], pe[:])
            for h in range(HL):
                nc.gpsimd.partition_broadcast(edb[:, h, :], edTh[h][0:1, :])
            # C-path factors
            esv = es_N[:].rearrange("p nb h -> p (nb h)")
            nc.vector.tensor_scalar(out=esn[:].rearrange("p nb h -> p (nb h)"),
                                    in0=esv, scalar1=-1.0, scalar2=None,
                                    op0=ALU.mult)
            edv = edN[:].rearrange("p nb h -> p (nb h)")
            nc.scalar.activation(a1[:].rearrange("p nb h -> p (nb h)"), edv,
                                 AF.Exp)
            nc.scalar.activation(a2[:].rearrange("p nb h -> p (nb h)"), edv,
                                 AF.Exp, scale=NEG_SLOPE)
            nc.vector.tensor_scalar(out=a2n[:].rearrange("p nb h -> p (nb h)"),
                                    in0=a2[:].rearrange("p nb h -> p (nb h)"),
                                    scalar1=-1.0, scalar2=None, op0=ALU.mult)
            nc.scalar.activation(esx1[:].rearrange("p nb h -> p (nb h)"), esv,
                                 AF.Exp)
            nc.scalar.activation(esx2[:].rearrange("p nb h -> p (nb h)"), esv,
                                 AF.Exp, scale=NEG_SLOPE)

        def build_xB12(h):
            """xB12 = [x0ext_h * exp(es_h) | x0ext_h * exp(.2 es_h)] (bf16)."""
            for v, esx in ((0, esx1), (1, esx2)):
                src = esx[:, :, h:h + 1]
                bcast = bass.AP(tensor=src.tensor, offset=src.offset,
                                ap=[src.ap[0], src.ap[1], [0, C + 1]])
                nc.vector.tensor_tensor(
                    out=xB12[:].rearrange("p nb (v c) -> p nb v c", v=2)[:, :, v, :],
                    in0=x0ext[:, :, h, :], in1=bcast, op=ALU.mult)

        def head_A(l, h, pagg):
            """ACT-path scores + aggregation for local head h into pagg."""
            for jb in range(NB):
                s = sc_s.tile([TB, N], F32, tag="s")
                nc.scalar.activation(s[:], edb[0:TB, h, :], AF.Prelu,
                                     bias=es_N[:, jb, h:h + 1], scale=1.0,
                                     alpha=NEG_SLOPE)
                t = sc_t.tile([TB, N], AGG_DT, tag="t")
                nc.scalar.activation(t[:], s[:], AF.Exp)
                P = sc_p.tile([TB, N], AGG_DT, tag="P")
                eng = nc.vector if jb % 2 == 0 else nc.gpsimd
                eng.tensor_tensor(out=P[:], in0=t[:], in1=gT[:, jb, :],
                                  op=ALU.mult)
                for (a, b_) in QF:
                    nc.tensor.matmul(pagg[:, a:b_], x0ext[:, jb, h, :],
                                     P[:, a:b_], start=(jb == 0),
                                     stop=(jb == NB - 1))

        def head_C(l, h, oTC, oTg):
            """No-ACT rank-1 path for local head h.
            oTC [66, N] <- M1|M2 stacked; oTg [33, N] <- mask stream."""
            build_xB12(h)
            for ih in range(2):
                ia, ib = ih * 1024, min((ih + 1) * 1024, N)
                m12 = psum_ag.tile([2 * (C + 1), 1024], F32, tag="aggC")
                for jb in range(NB):
                    Ht = sc_t.tile([TB, 1024], BF16, tag="Ht")
                    nc.vector.tensor_scalar(out=Ht[:, 0:ib - ia],
                                            in0=edb[0:TB, h, ia:ib],
                                            scalar1=esn[:, jb, h:h + 1],
                                            scalar2=None, op0=ALU.is_ge)
                    G1 = sc_p.tile([TB, 1024], BF16, tag="G1")
                    nc.vector.tensor_tensor(out=G1[:, 0:ib - ia],
                                            in0=Ht[:, 0:ib - ia],
                                            in1=gT[:, jb, ia:ib], op=ALU.mult)
                    for (a, b_) in QH[ih]:
                        nc.tensor.matmul(m12[:, a - ia:b_ - ia],
                                         xB12[:, jb, :], G1[:, a - ia:b_ - ia],
                                         start=(jb == 0), stop=(jb == NB - 1))
                nc.vector.tensor_copy(oTC[:, ia:ib], m12[:, 0:ib - ia])
            for ih in range(2):
                ia, ib = ih * 1024, min((ih + 1) * 1024, N)
                mg = psum_ag.tile([C + 1, 1024], F32, tag="aggC")
                for jb in range(NB):
                    for (a, b_) in QH[ih]:
                        nc.tensor.matmul(mg[:, a - ia:b_ - ia],
                                         xB12[:, jb, C + 1:2 * (C + 1)],
                                         gT[:, jb, a:b_],
                                         start=(jb == 0), stop=(jb == NB - 1))
                nc.scalar.copy(oTg[:, ia:ib], mg[:, 0:ib - ia])

        def head_tail(l, h, hout_of_nb):
            """Per-head epilogue: normalize by Z and emit per-layer halves."""
            for nb in range(NB):
                hout = hout_of_nb(nb)
                z = work.tile([TB, 1], F32, tag="z")
                if l == 0:
                    nc.vector.tensor_copy(z[:], hout[:, C:C + 1])
                else:
                    nc.vector.tensor_scalar(out=z[:], in0=hout[:, C:C + 1],
                                            scalar1=float(H), scalar2=None,
                                            op0=ALU.mult)
                rz = work.tile([TB, 1], F32, tag="rz")
                nc.vector.reciprocal(rz[:], z[:])
                if l == 0:
                    h0p = work.tile([TB, C], F32, tag="h0p")
                    nc.vector.tensor_scalar(out=h0p[:], in0=hout[:, 0:C],
                                            scalar1=rz[:, 0:1], scalar2=None,
                                            op0=ALU.mult)
                    nc.sync.dma_start(out=ag_in_d[ts(nb, TB), ts(h, C)],
                                      in_=h0p[:])
                else:
                    if h == 0:
                        nc.vector.tensor_scalar(out=hpacc[:, nb, :],
                                                in0=hout[:, 0:C],
                                                scalar1=rz[:, 0:1],
                                                scalar2=None, op0=ALU.mult)
                    else:
                        nc.vector.scalar_tensor_tensor(
                            out=hpacc[:, nb, :], in0=hout[:, 0:C],
                            scalar=rz[:, 0:1], in1=hpacc[:, nb, :],
                            op0=ALU.mult, op1=ALU.add)
                        nc.sync.dma_start(out=ar_in_d[ts(nb, TB), :],
                                          in_=hpacc[:, nb, :])

        def tail_A(l, h, pagg):
            def hout_of_nb(nb):
                oTn = work.tile([C + 1, TB], F32, tag="oTn")
                if nb % 2 == 0:
                    nc.scalar.copy(oTn[:], pagg[:, ts(nb, TB)])
                else:
                    nc.vector.tensor_copy(oTn[:], pagg[:, ts(nb, TB)])
                pt2 = psum.tile([TB, C + 1], F32, tag="ps")
                nc.tensor.transpose(pt2[:], oTn[:], ident[0:C + 1, 0:C + 1])
                return pt2
            head_tail(l, h, hout_of_nb)

        def tail_C(l, h, oTC, oTg):
            def hout_of_nb(nb):
                ptc = psum.tile([TB, 3 * (C + 1)], F32, tag="ps")
                nc.tensor.matmul(ptc[:, 0:2 * (C + 1)], oTC[:, ts(nb, TB)],
                                 ident[0:2 * (C + 1), 0:2 * (C + 1)],
                                 is_transpose=True, start=True, stop=False)
                nc.tensor.matmul(ptc[:, 2 * (C + 1):], oTg[:, ts(nb, TB)],
                                 ident[0:C + 1, 0:C + 1],
                                 is_transpose=True, start=False, stop=True)
                u1 = work.tile([TB, C + 1], F32, tag="u1")
                nc.scalar.activation(u1[:], ptc[:, 0:C + 1], AF.Copy,
                                     scale=a1[:, nb, h:h + 1])
                u2 = work.tile([TB, C + 1], F32, tag="u2")
                nc.scalar.activation(u2[:], ptc[:, C + 1:2 * (C + 1)], AF.Copy,
                                     scale=a2n[:, nb, h:h + 1])
                acc = work.tile([TB, C + 1], F32, tag="acc2")
                nc.vector.tensor_tensor(out=acc[:], in0=u1[:], in1=u2[:],
                                        op=ALU.add)
                hc = work.tile([TB, C + 1], F32, tag="hc")
                nc.vector.scalar_tensor_tensor(out=hc[:],
                                               in0=ptc[:, 2 * (C + 1):],
                                               scalar=a2[:, nb, h:h + 1],
                                               in1=acc[:], op0=ALU.mult,
                                               op1=ALU.add)
                return hc
            head_tail(l, h, hout_of_nb)

        def gat_layer(l, hT_in, skip_px=False):
            stage1(l, hT_in, skip_px=skip_px)
            for h in range(HL):
                if h in c_heads:
                    oTC = ot_p.tile([2 * (C + 1), N], F32, tag="oTC")
                    oTg = ot_p.tile([C + 1, N], F32, tag="oTg")
                    head_C(l, h, oTC, oTg)
                    tail_C(l, h, oTC, oTg)
                else:
                    pagg = psum_ag.tile([C + 1, 2048], F32, tag="aggA")
                    head_A(l, h, pagg)
                    tail_A(l, h, pagg)

        # ================= layer 0 =================
        gat_layer(0, hT)
        if dumps:
            nc.sync.dma_start(out=dbg["agin"][:], in_=ag_in_d[:])
        if no_cc:
            nc.gpsimd.dma_start(out=ag_out_d[0], in_=ag_in_d[:])
            nc.gpsimd.dma_start(out=ag_out_d[1], in_=ag_in_d[:])
        else:
            nc.gpsimd.collective_compute(
                "AllGather", ALU.bypass, replica_groups=PAIRS,
                ins=[ag_in_d[:].opt()], outs=[ag_out_d[:].opt()])

        # h_new = h + elu(h0 + bias0); overwrite hN, build hT1
        for nb in range(NB):
            h0f = work.tile([TB, D], F32, tag="h0f")
            nc.sync.dma_start(out=h0f[:, 0:HL * C], in_=ag_out_d[0, ts(nb, TB), :])
            nc.sync.dma_start(out=h0f[:, HL * C:D], in_=ag_out_d[1, ts(nb, TB), :])
            h0b = work.tile([TB, D], F32, tag="h0b")
            nc.vector.tensor_tensor(out=h0b[:], in0=h0f[:], in1=b0b[0:TB, :],
                                    op=ALU.add)
            r2 = work.tile([TB, D], F32, tag="r2")
            nc.scalar.activation(r2[:], h0b[:], AF.Relu, scale=-1.0)
            ex = work.tile([TB, D], F32, tag="ex")
            nc.scalar.activation(ex[:], r2[:], AF.Exp, scale=-1.0)
            acc = work.tile([TB, D], F32, tag="acc")
            nc.vector.tensor_tensor(out=acc[:], in0=hN[:, nb, :], in1=h0b[:],
                                    op=ALU.add)
            nc.vector.tensor_tensor(out=acc[:], in0=acc[:], in1=r2[:], op=ALU.add)
            nc.vector.scalar_tensor_tensor(out=hN[:, nb, :], in0=ex[:],
                                           scalar=-1.0, in1=acc[:],
                                           op0=ALU.add, op1=ALU.add)
            pt1 = psum.tile([D, TB], F32, tag="ps")
            nc.tensor.transpose(pt1[:], hN[:, nb, :], ident[0:TB, 0:TB])
            nc.scalar.copy(hT[:, ts(nb, TB)], pt1[:])
            stage1_block(1, hT, nb)
        if dumps:
            nc.sync.dma_start(out=dbg["hN2"][:], in_=hN[:])

        # ================= layer 1 =================
        gat_layer(1, hT, skip_px=True)
        if dumps:
            nc.sync.dma_start(out=dbg["arin"][:], in_=ar_in_d[:])
        if no_cc:
            nc.gpsimd.dma_start(out=ar_out_d[:], in_=ar_in_d[:])
        else:
            nc.gpsimd.collective_compute(
                "AllReduce", ALU.add, replica_groups=PAIRS,
                ins=[ar_in_d[:].opt()], outs=[ar_out_d[:].opt()])
        if dumps:
            nc.sync.dma_start(out=dbg["arout"][:], in_=ar_out_d[:])

        # final projection: out = h1 @ W_out + (bias1 @ W_out + b_out)
        for nb in range(NB):
            nc.sync.dma_start(out=h1aug[:, nb, 0:C], in_=ar_out_d[ts(nb, TB), :])
            ptc = psum.tile([C + 1, TB], F32, tag="ps")
            nc.tensor.transpose(ptc[:], h1aug[:, nb, :], ident[0:TB, 0:TB])
            lhs = work.tile([C + 1, TB], F32, tag="lhs")
            nc.vector.tensor_copy(lhs[:], ptc[:])
            po = psum.tile([TB, D], F32, tag="ps")
            nc.tensor.matmul(po[:], lhs[:], Woa[:], start=True, stop=True)
            ob = work.tile([TB, D], F32, tag="ob")
            nc.scalar.copy(ob[:], po[:])
            nc.sync.dma_start(out=out_d[ts(nb, TB), :], in_=ob[:])

    nc.compile()
    return nc


def _host_prep(inputs):
    """Build the 8 per-core input maps (pure numpy, not in HW time)."""
    f32 = np.float32
    x = np.asarray(inputs["x_alpha"], f32)[:, -1]            # [B, N, F_IN]
    sg = np.asarray(inputs["sector_graph"], f32)
    W_in = np.asarray(inputs["W_in"], f32)
    b_in = np.asarray(inputs["b_in"], f32)
    ln_g = np.asarray(inputs["ln_g"], f32)
    ln_b = np.asarray(inputs["ln_b"], f32)
    W0 = np.asarray(inputs["W0"], f32)
    as0 = np.asarray(inputs["as0"], f32)
    ad0 = np.asarray(inputs["ad0"], f32)
    bias0 = np.asarray(inputs["bias0"], f32)
    W1 = np.asarray(inputs["W1"], f32)
    as1 = np.asarray(inputs["as1"], f32)
    ad1 = np.asarray(inputs["ad1"], f32)
    bias1 = np.asarray(inputs["bias1"], f32)
    W_out = np.asarray(inputs["W_out"], f32)
    b_out = np.asarray(inputs["b_out"], f32)

    Wia = np.concatenate([W_in, b_in[None, :]], axis=0)       # [159, 128]
    lngb = np.ascontiguousarray(np.tile(ln_g[None, :], (128, 1)))
    lnbb = np.ascontiguousarray(np.tile(ln_b[None, :], (128, 1)))
    b0b = np.ascontiguousarray(np.tile(bias0[None, :], (128, 1)))
    Woa = np.concatenate([W_out, (bias1 @ W_out + b_out)[None, :]], axis=0)

    eye = np.eye(N, dtype=bool)
    in_maps = []
    for c in range(8):
        b = c // 2
        hp = c % 2
        heads = [2 * hp, 2 * hp + 1]
        xa = np.concatenate([x[b], np.ones((N, 1), f32)], axis=1)  # [N, 159]
        xT = np.ascontiguousarray(xa.T)                            # [159, N]
        mask = (sg[b] > 0) | eye
        gT = np.ascontiguousarray(mask.T).astype(ml_dtypes.bfloat16)

        def pack_p(W, as_, ad_):
            cols = [W[:, h * C:(h + 1) * C] for h in heads]
            cols += [(W[:, h * C:(h + 1) * C] @ as_[h])[:, None] for h in heads]
            cols += [(W[:, h * C:(h + 1) * C] @ ad_[h])[:, None] for h in heads]
            return np.ascontiguousarray(np.concatenate(cols, axis=1))

        def pack_d(W, ad_):
            cols = [(W[:, h * C:(h + 1) * C] @ ad_[h])[:, None] for h in heads]
            return np.ascontiguousarray(np.concatenate(cols, axis=1))

        in_maps.append({
            "xTa": np.ascontiguousarray(xT[0:KA]),
            "xTb": np.ascontiguousarray(xT[KA:FA]),
            "gT": gT,
            "WiaA": np.ascontiguousarray(Wia[0:KA]),
            "WiaB": np.ascontiguousarray(Wia[KA:FA]),
            "Wp0": pack_p(W0, as0, ad0), "Wp1": pack_p(W1, as1, ad1),
            "Wad0": pack_d(W0, ad0), "Wad1": pack_d(W1, ad1),
            "lngb": lngb, "lnbb": lnbb, "b0b": b0b,
            "Woa": np.ascontiguousarray(Woa),
        })
    return in_maps


def kernel(**inputs):
    if "nc" not in _CACHE:
        _CACHE["nc"] = _build_program()
    nc = _CACHE["nc"]
    in_maps = _host_prep(inputs)
    res = run_bass_kernel_spmd(nc, in_maps, list(range(8)),
                               **_CACHE.get("run_kwargs", {}))
    _CACHE["last_results"] = res
    out = np.empty((B, N, D), np.float32)
    for b in range(B):
        out[b] = res.results[2 * b]["out"]
    return out

